# revision 35
# baseline (speedup 1.0000x reference)
"""Distributed Trainium2 Bass kernel for a pre-LN single attention block.

Reference computation (per problem):
    xn  = LayerNorm(x) * ln_scale + ln_bias          x: [4096, 1024]
    qkv = xn @ w_qkv                                 w_qkv: [1024, 3072]
    q, k, v = split(qkv); heads = 16, dim_head = 64
    sim = einsum('ihd,jhd->ijh', q, k) / sqrt(1024)
    attn = softmax(sim, axis=j)
    out = einsum('ijh,jhd->ihd', attn, v) @ w_out + b_out

Sharding: sequence-parallel over the 8 NeuronCores (512 rows each). Each core
LayerNorms its row shard, computes its qkv^T, AllGathers K^T (its flight
hidden under V+Q compute) then V (hidden under early score groups), runs
attention over the full key range for its 512 query rows, and applies the
output projection locally. No all-reduce is needed: each core owns disjoint
output rows.

Attention inner loop: per head pair, 32 j-tiles x 2 heads of score matmuls
stream through a 2-deep PSUM ring of 2-tile groups; exp batches run on ACT;
the attn*V matmuls for group g are emitted 2 groups late (AV_DEFER=2) so the
in-order PE never waits on ACT (1 group of slack measured ~serial on HW),
and av PSUM is double-buffered across pairs (AV_BUFS=2) so pair boundaries
don't stall on the previous pair's sums/avn drain reads.
Per-pair normalization work is reduced to four DVE row-copies (payload +
sumexp rows); all reciprocal/broadcast/multiply work is batched into the
two projection stages where it pipelines against proj matmuls.

Key device-side layout decisions (bf16 compute / f32 accumulate):
  zT      [dim, rows]     LN output, transposed via PE so both qkv matmul
                          operands have the contraction dim on partitions
  qkvT    [3072, rows]    = W^T zT (ln_scale folded into W on the host)
  scores  ST[j, i] via lhsT=K_h^T — softmax reduction lands on the free axis
          of the attn*V matmul output instead of partitions
  vaug    [j, 24*64] per j-tile: [v_e | ones | v_o] per pair; the attn*V
          lhsT slice [v_h | ones] / [ones | v_h] is contiguous (matmul lhsT
          APs allow only one free dim) and emits sumexp rows for free
  out     yT [1024, 512]  transposed output shard; host transposes + concats
"""

import os

import numpy as np

import concourse.bass as bass
import concourse.mybir as mybir
import concourse.tile as tile
from concourse import bacc
from concourse.bass_utils import run_bass_kernel_spmd
from concourse.masks import make_identity


F32 = mybir.dt.float32
BF16 = mybir.dt.bfloat16
I16 = mybir.dt.int16
AF = mybir.ActivationFunctionType
ALU = mybir.AluOpType

NC_CORES = 8
SEQ = 4096
DIM = 1024
HEADS = 16
DH = 64
S = SEQ // NC_CORES          # 512 query rows per core
RT = S // 128                # 4 row tiles per core
KT = DIM // 128              # 8 contraction tiles over dim
JT = SEQ // 128              # 32 key tiles
VW = 24 * DH                 # vaug width: per pair [v_even | ones | v_odd]
NB = VW // DH                # 24 64-col blocks per vaug tile
LN_EPS = 1e-6
SOFTMAX_SCALE = float(DIM) ** -0.5

ST_GROUP = int(os.environ.get("ATTN_ST_GROUP", "2"))  # j-tile-halves per exp batch
AV_BUFS = int(os.environ.get("ATTN_AV_BUFS", "2"))    # av PSUM buffering
# PSUM budget: N_GROUPS*ST_GROUP (score ring) + 2*AV_BUFS (av) = 8 banks.
# With AV_DEFER=2: SG2+AV2 (ring of 2, av double-buffered across pairs)
# beats SG2+AV1 by ~180us on the r1-floor protocol — pair p+1's first av
# matmuls stop waiting on pair p's sums/avn PSUM reads at the boundary.
N_GROUPS = (8 - 2 * AV_BUFS) // ST_GROUP
# exp routing pattern: 1=ACT exp, 0=DVE Schraudolph bf16-bit-trick exp.
# Measured: DVE-split patterns ("10", "110") are timing-neutral at best on
# this fabric and cost accuracy (~1.1-1.3e-2 vs 4.8e-3), so default all-ACT.
EXP_PATTERN = [int(c) for c in os.environ.get("ATTN_EXP_PATTERN", "1")]
# output-projection split point: pairs [0, PROJ_SPLIT) normalized + projected
# mid-attention (after pair PROJ_SPLIT-1 finishes, while the remaining pairs'
# attention runs). Also bounds live sums tiles (SBUF) to max(PROJ_SPLIT, 8-
# PROJ_SPLIT).
PROJ_SPLIT = int(os.environ.get("ATTN_PROJ_SPLIT", "4"))
# attn*V emission deferral depth (groups of lookahead in the PE stream).
# 1 leaves the HW pipeline ~serial (exp -> av -> next scores, ~2.7us/group);
# 2 lets ACT run far enough ahead that PE never waits (measured 513 -> 199us).
AV_DEFER = int(os.environ.get("ATTN_AV_DEFER", "2"))

# Schraudolph bf16 exp: bf16_bits(exp(s/32)) ~= int16(s*(128/ln2/32) + B)
# C calibrated for min RMS rel err (~1.9%) assuming round-to-nearest convert.
SCHRAU_C = float(os.environ.get("ATTN_SCHRAU_C", "8"))
SCHRAU_A = (128.0 / np.log(2.0)) * SOFTMAX_SCALE
SCHRAU_B = 127.0 * 128.0 - SCHRAU_C


def build_graph(sim=False, debug=False, reps=1, phases=None):
    phases = phases or os.environ.get("ATTN_PHASES", "all")
    nc = bacc.Bacc(
        "TRN2",
        target_bir_lowering=False,
        debug=False,
        num_devices=NC_CORES,
    )

    x_ext = nc.declare_dram_parameter("x", [S, DIM], F32, isOutput=False)
    wq_ext = nc.declare_dram_parameter("wq", [DIM, 3 * DIM], BF16, isOutput=False)
    cv_ext = nc.declare_dram_parameter("cvec", [3 * DIM], F32, isOutput=False)
    wo_ext = nc.declare_dram_parameter("wo", [DIM, DIM], BF16, isOutput=False)
    out_ext = nc.declare_dram_parameter("out", [DIM, S], F32, isOutput=True)

    x_ap = x_ext.ap()
    wq_ap = wq_ext.ap()
    cv_ap = cv_ext.ap()
    wo_ap = wo_ext.ap()
    out_ap = out_ext.ap()

    groups = [list(range(NC_CORES))]
    KSZ = DIM * S                # elements in the K^T bounce
    VSZ = S * VW                 # elements in the V bounce (vaug layout)

    with tile.TileContext(nc) as tc:
      for _rep in range(reps):
        with (
            tc.tile_pool(name="singles", bufs=1) as singles,
            tc.tile_pool(name="dram", bufs=1, space="DRAM") as dram,
        ):
            # ---- constants ----
            ident = singles.tile([128, 128], BF16)
            make_identity(nc, ident)
            eps_sb = singles.tile([128, 1], F32)
            nc.vector.memset(eps_sb, LN_EPS)
            if reps > 1:
                # serialize reps: eps (hence everything) depends on the
                # previous rep's final output write — makes the reps-delta
                # measure single-shot latency instead of pipelined throughput
                dep_t = singles.tile([1, 64], F32, name="dep")
                nc.sync.dma_start(dep_t[0:1, :], out_ap[0:1, 0:64])
                nc.vector.tensor_scalar_mul(dep_t[0:1, 0:1], dep_t[0:1, 0:1], 0.0)
                nc.vector.tensor_tensor(
                    eps_sb[0:1, :], eps_sb[0:1, :], dep_t[0:1, 0:1], ALU.add
                )
            ones_sb = singles.tile([128, 64], BF16)
            nc.gpsimd.memset(ones_sb, 1.0)
            cv_sb = singles.tile([128, 24], F32)
            nc.sync.dma_start(cv_sb, cv_ap.rearrange("(m p) -> p m", p=128))

            # ---- collective buffers (split K / V AllGathers: K's flight
            # hides under V+Q compute; kpair DMAs gate on K-AG only) ----
            # NOTE: Shared-addr-space collective outputs hang on this axon
            # terminal; Local works (verified with a minimal 8-core AllGather).
            k_bounce = dram.tile([KSZ], BF16)
            k_all = dram.tile([NC_CORES, KSZ], BF16)
            v_bounce = dram.tile([VSZ], BF16)
            v_all = dram.tile([NC_CORES, VSZ], BF16)

            # ---- persistent SBUF activations ----
            qT_sb = singles.tile([128, KT, S], BF16)     # q^T   (m-tiles 0-7)
            avn_sb = singles.tile([128, KT, S], BF16)    # normalized attn out ^T

            # ============ Stage A: LayerNorm + transpose ============
            stage_ab = tc.tile_pool(name="stage_ab", bufs=1)
            ab = stage_ab.__enter__()
            # local V in vaug layout [v_e | ones | v_o] per pair; the whole
            # tile (ones included) is the AG payload
            vA_sb = ab.tile([128, RT, VW], BF16)
            nc.gpsimd.memset(
                vA_sb.rearrange("p r (b e) -> p r b e", e=DH)[:, :, 1::3, :], 1.0
            )
            wq_sb = ab.tile([128, KT, 3 * DIM], BF16)
            for kt in range(KT):
                nc.sync.dma_start(
                    wq_sb[:, kt], wq_ap.rearrange("(k p) m -> p k m", p=128)[:, kt]
                )
            zT_sb = ab.tile([128, KT, S], BF16)          # LN(x)^T
            kT_sb = ab.tile([128, KT, S], BF16)          # k^T   (m-tiles 8-15)

            with (
                tc.tile_pool(name="ln", bufs=2) as ln_pool,
                tc.tile_pool(name="ln_ps", bufs=4, space="PSUM") as ln_ps,
            ):
                for rt in range(RT):
                    x_t = ln_pool.tile([128, DIM], F32, tag="x")
                    nc.sync.dma_start(
                        x_t, x_ap.rearrange("(t p) d -> t p d", p=128)[rt]
                    )
                    stats = ln_pool.tile([128, 2, 6], F32, tag="stats")
                    for sg in range(2):
                        nc.vector.bn_stats(stats[:, sg], x_t[:, 512 * sg:512 * (sg + 1)])
                    mv = ln_pool.tile([128, 2], F32, tag="mv")
                    nc.vector.bn_aggr(mv, stats)
                    std = ln_pool.tile([128, 1], F32, tag="std")
                    nc.scalar.activation(std, mv[:, 1:2], AF.Sqrt, bias=eps_sb)
                    rstd = ln_pool.tile([128, 1], F32, tag="rstd")
                    nc.vector.reciprocal(rstd, std)
                    z_t = ln_pool.tile([128, DIM], BF16, tag="z")
                    nc.vector.tensor_scalar(
                        z_t, x_t, mv[:, 0:1], rstd,
                        op0=ALU.subtract, op1=ALU.mult,
                    )
                    for kt in range(KT):
                        ps_t = ln_ps.tile([128, 128], BF16, tag="tps", name="tps")
                        nc.tensor.transpose(ps_t, z_t[:, 128 * kt:128 * (kt + 1)], ident)
                        nc.vector.tensor_copy(
                            out=zT_sb[:, kt, 128 * rt:128 * (rt + 1)], in_=ps_t
                        )

            # ============ Stage B: qkvT = W^T z^T; K, V, AllGather, Q ======
            with (
                tc.tile_pool(name="qkv_ps", bufs=4, space="PSUM") as qkv_ps,
                tc.tile_pool(name="vtr", bufs=3) as vtr_pool,
                tc.tile_pool(name="vtr_ps", bufs=4, space="PSUM") as vtr_ps,
            ):
                def qkv_mtile(m, dst_sb, dst_slot):
                    ps = qkv_ps.tile([128, S], F32, tag="qkvps", name="qkvps")
                    for kt in range(KT):
                        nc.tensor.matmul(
                            ps,
                            lhsT=wq_sb[:, kt, 128 * m:128 * (m + 1)],
                            rhs=zT_sb[:, kt, :],
                            start=(kt == 0),
                            stop=(kt == KT - 1),
                        )
                    nc.vector.tensor_scalar_add(
                        out=dst_sb[:, dst_slot, :], in0=ps, scalar1=cv_sb[:, m:m + 1]
                    )

                # K block (qkv columns 1024-2047)
                for mk in range(KT):
                    qkv_mtile(8 + mk, kT_sb, mk)
                nc.sync.dma_start(
                    k_bounce[:].rearrange("(k p i) -> p k i", p=128, i=S),
                    kT_sb,
                )
                if sim:
                    for c in range(NC_CORES):
                        nc.sync.dma_start(
                            k_all[c].rearrange("(k p i) -> p k i", p=128, i=S),
                            kT_sb,
                        )
                else:
                    nc.gpsimd.collective_compute(
                        "AllGather", ALU.bypass, replica_groups=groups,
                        ins=[k_bounce[:].opt()], outs=[k_all[:].opt()],
                    )

                # V block (qkv columns 2048-3071) -> natural layout in vA_sb
                for mv_ in range(KT):
                    vT_t = vtr_pool.tile([128, S], BF16, tag="vT", name="vT")
                    ps = qkv_ps.tile([128, S], F32, tag="qkvps", name="qkvps")
                    m = 16 + mv_
                    for kt in range(KT):
                        nc.tensor.matmul(
                            ps,
                            lhsT=wq_sb[:, kt, 128 * m:128 * (m + 1)],
                            rhs=zT_sb[:, kt, :],
                            start=(kt == 0),
                            stop=(kt == KT - 1),
                        )
                    nc.vector.tensor_scalar_add(
                        out=vT_t, in0=ps, scalar1=cv_sb[:, m:m + 1]
                    )
                    # heads 2*mv_, 2*mv_+1 -> v blocks 3*mv_ and 3*mv_+2
                    for rt in range(RT):
                        ps_t = vtr_ps.tile([128, 128], BF16, tag="vtps", name="vtps")
                        nc.tensor.transpose(ps_t, vT_t[:, 128 * rt:128 * (rt + 1)], ident)
                        nc.vector.tensor_copy(
                            out=vA_sb.rearrange("p r (b e) -> p r b e", e=DH)[
                                :, rt, 3 * mv_:3 * mv_ + 3:2, :
                            ],
                            in_=ps_t.rearrange("p (b e) -> p b e", e=DH),
                        )
                nc.sync.dma_start(
                    v_bounce[:].rearrange("(r p v) -> p r v", p=128, v=VW),
                    vA_sb,
                )
                if sim:
                    for c in range(NC_CORES):
                        nc.sync.dma_start(
                            v_all[c].rearrange("(r p v) -> p r v", p=128, v=VW),
                            vA_sb,
                        )
                else:
                    nc.gpsimd.collective_compute(
                        "AllGather", ALU.bypass, replica_groups=groups,
                        ins=[v_bounce[:].opt()], outs=[v_all[:].opt()],
                    )

                # Q block (qkv columns 0-1023) — during the AllGather flight
                for mq in range(KT):
                    qkv_mtile(mq, qT_sb, mq)
            stage_ab.__exit__(None, None, None)

            # output-projection weights: loaded during attention
            wo_sb = singles.tile([128, KT, DIM], BF16)
            nc.sync.dma_start(wo_sb, wo_ap.rearrange("(k p) n -> p k n", p=128))
            # projection partial accumulator (pairs 0..PROJ_SPLIT-1); bf16 —
            # adds ~0.3% rounding on the partial sum, well inside budget
            yh_sb = singles.tile([128, KT, S], BF16)

            # views of the gathered buffers
            kall = k_all[:].rearrange("c (k p i) -> p c k i", p=128, i=S)
            vall = v_all[:].rearrange("c (r p v) -> p c r v", p=128, v=VW)

            # ============ Stage C: attention, head pairs ============
            if phases == "head":
                # consume the collective outputs so their completion is timed
                with tc.tile_pool(name="hddbg", bufs=1) as hd:
                    t1 = hd.tile([128, S], BF16, name="hk")
                    nc.sync.dma_start(t1, kall[:, NC_CORES - 1, 0, :])
                    t2 = hd.tile([128, S], F32, name="hk2")
                    nc.vector.tensor_copy(out=t2, in_=t1)
                    nc.sync.dma_start(out_ap[0:128, :], t2)
                    t3 = hd.tile([128, S], BF16, name="hv")
                    nc.sync.dma_start(t3, vall[:, NC_CORES - 1, 0, 0:S])
                    t4 = hd.tile([128, S], F32, name="hv2")
                    nc.vector.tensor_copy(out=t4, in_=t3)
                    nc.sync.dma_start(out_ap[128:256, :], t4)
            if phases != "head":
              with tc.tile_pool(name="vaug", bufs=1) as vaug_pool, \
                 tc.tile_pool(name="kpair", bufs=2) as kpair_pool:
                # first head-pair's K tile loads BEFORE the vaug stream
                kpair0 = kpair_pool.tile([128, NC_CORES, S], BF16, tag="kpair",
                                         name="kpair0")
                nc.sync.dma_start(kpair0, kall[:, :, 0, :])
                # vaug tiles arrive attn*V-ready (ones blocks in the payload)
                vaug = []
                for jt in range(JT):
                    vt = vaug_pool.tile([128, VW], BF16, name=f"vaug{jt}")
                    nc.sync.dma_start(vt, vall[:, jt // RT, jt % RT, :])
                    vaug.append(vt)

                with (
                    tc.tile_pool(name="st_ps", bufs=1, space="PSUM") as st_ps_pool,
                    tc.tile_pool(name="av_ps", bufs=AV_BUFS, space="PSUM") as av_ps_pool,
                    tc.tile_pool(name="expt", bufs=int(os.environ.get("ATTN_EXPT_BUFS", "8"))) as expt_pool,
                    tc.tile_pool(name="norm", bufs=2) as norm_pool,
                    tc.tile_pool(name="sums", bufs=max(PROJ_SPLIT, HEADS // 2 - PROJ_SPLIT)) as sums_pool,
                ):
                    import itertools
                    exp_rr = itertools.cycle(EXP_PATTERN)
                    last_expt = [None]      # for phases=="exp" consume
                    sums_list = []          # per-pair [2, S] sumexp rows
                    for p in range(HEADS // 2):
                        if p == 0:
                            kpair = kpair0
                        else:
                            kpair = kpair_pool.tile([128, NC_CORES, S], BF16,
                                                    tag="kpair", name="kpair")
                            nc.sync.dma_start(kpair, kall[:, :, p, :])
                        av_A = av_ps_pool.tile([128, S], F32, tag="avA", name="avA")
                        av_B = av_ps_pool.tile([128, S], F32, tag="avB", name="avB")
                        av_AB = (av_A, av_B)

                        # pair-interleaved score tile stream, PSUM ring
                        st_group = [None] * N_GROUPS
                        pending = []  # (hh, jt, group, col) awaiting exp+av
                        # attn*V for group g is emitted AV_DEFER groups LATE
                        # so later groups' (independent) score matmuls precede
                        # it in the in-order PE stream; depth 1 still measures
                        # ~serial exp->av->scores on HW, depth 2 gives ACT
                        # enough runway that the PE never waits.
                        av_defer = []  # (expt, cols) awaiting emission

                        def drain_av(keep, p=p, av_AB=av_AB, av_defer=av_defer):
                            while len(av_defer) > keep:
                                expt, cols = av_defer.pop(0)
                                if phases == "exp":
                                    continue
                                for (hh, jt, gg, col) in cols:
                                    # even head: [v_h|ones]; odd: [ones|v_h]
                                    off = 192 * p + 64 * hh
                                    nc.tensor.matmul(
                                        av_AB[hh],
                                        lhsT=vaug[jt][:, off:off + 128],
                                        rhs=expt[:, 512 * col:512 * (col + 1)],
                                        start=(jt == 0),
                                        stop=(jt == JT - 1),
                                    )

                        def flush_group(g, pending=pending,
                                        st_group=st_group, av_defer=av_defer):
                            cols = [t for t in pending if t[2] == g]
                            ncols = len(cols)
                            expt = expt_pool.tile(
                                [128, ST_GROUP * 512], BF16, tag="expt", name="expt"
                            )
                            last_expt[0] = expt
                            if next(exp_rr) == 0:
                                nc.vector.tensor_scalar(
                                    expt[:, :512 * ncols].bitcast(I16),
                                    st_group[g][:, :512 * ncols],
                                    SCHRAU_A, SCHRAU_B,
                                    op0=ALU.mult, op1=ALU.add,
                                )
                            else:
                                nc.scalar.activation(
                                    expt[:, :512 * ncols],
                                    st_group[g][:, :512 * ncols], AF.Exp,
                                    scale=SOFTMAX_SCALE,
                                )
                            pending[:] = [t for t in pending if t[2] != g]
                            av_defer.append((expt, cols))
                            drain_av(keep=AV_DEFER)


                        slot = 0
                        nflush = 0
                        for jt in range(JT):
                            for hh in range(2):
                                g, col = slot // ST_GROUP, slot % ST_GROUP
                                if col == 0:
                                    st_group[g] = st_ps_pool.tile(
                                        [128, ST_GROUP * 512], F32,
                                        tag=f"stg{g}", name=f"stg{g}"
                                    )
                                c, ioff = jt // 4, (jt % 4) * 128
                                nc.tensor.matmul(
                                    st_group[g][:, 512 * col:512 * (col + 1)],
                                    lhsT=kpair[64 * hh:64 * (hh + 1), c,
                                               ioff:ioff + 128],
                                    rhs=qT_sb[64 * hh:64 * (hh + 1), p, :],
                                    start=True, stop=True,
                                )
                                pending.append((hh, jt, g, col))
                                slot += 1
                                if slot % ST_GROUP == 0:
                                    if phases == "scores":
                                        pending.clear()
                                    else:
                                        flush_group(g)
                                        nflush += 1
                                    slot = slot % (N_GROUPS * ST_GROUP)
                        if pending:
                            if phases == "scores":
                                pending.clear()
                            else:
                                flush_group((slot - 1) // ST_GROUP % N_GROUPS)
                        if phases != "scores":
                            drain_av(keep=0)
                        if phases == "scores":
                            if p == HEADS // 2 - 1:
                                sc_t = norm_pool.tile([128, S], F32, tag="tmpr",
                                                      name="scconsume")
                                nc.vector.tensor_copy(
                                    out=sc_t, in_=st_group[0][:, 0:S]
                                )
                                nc.sync.dma_start(out_ap[0:128, :], sc_t)
                            continue
                        if phases == "exp":
                            if p == HEADS // 2 - 1:
                                sc_t = norm_pool.tile([128, S], F32, tag="tmpr",
                                                      name="expconsume")
                                nc.vector.tensor_copy(
                                    out=sc_t, in_=last_expt[0][:, 0:S]
                                )
                                nc.sync.dma_start(out_ap[0:128, :], sc_t)
                            continue
                        if phases == "av":
                            if p == HEADS // 2 - 1:
                                sc_t = norm_pool.tile([128, S], F32, tag="tmpr",
                                                      name="avconsume")
                                nc.vector.tensor_copy(out=sc_t, in_=av_A)
                                nc.sync.dma_start(out_ap[0:128, :], sc_t)
                            continue

                        # cheap per-pair drain: stash the two sumexp rows and
                        # the unnormalized payload; all reciprocal/broadcast/
                        # normalize work is batched into the projection stage
                        # (the old per-pair recip->gpsimd->PE-bc->mult chain
                        # serialized ~12us/pair on HW)
                        sums2 = sums_pool.tile([1, 2 * S], F32, name="sums2")
                        nc.vector.tensor_copy(out=sums2[0:1, 0:S],
                                              in_=av_A[64:65, :])
                        nc.vector.tensor_copy(out=sums2[0:1, S:2 * S],
                                              in_=av_B[0:1, :])
                        nc.vector.tensor_copy(out=avn_sb[0:64, p, :],
                                              in_=av_A[0:64, :])
                        nc.vector.tensor_copy(out=avn_sb[64:128, p, :],
                                              in_=av_B[64:128, :])
                        sums_list.append(sums2)

                        def finish_pairs(plist):
                            # batched normalization: avn[:, p] *= bc(1/sums)
                            for pp in plist:
                                s2 = sums_list[pp]
                                r2 = norm_pool.tile([1, 2 * S], F32, tag="r2",
                                                    name="r2")
                                nc.vector.reciprocal(r2, s2)
                                r2b = norm_pool.tile([1, 2 * S], BF16, tag="r2b",
                                                     name="r2b")
                                nc.vector.tensor_copy(out=r2b, in_=r2)
                                bc = st_ps_pool.tile(
                                    [128, S], F32,
                                    tag=f"stg{pp % N_GROUPS}", name="bc"
                                )
                                for hh in range(2):
                                    nc.tensor.matmul(
                                        bc[64 * hh:64 * (hh + 1), :],
                                        lhsT=ones_sb[0:1, :],
                                        rhs=r2b[0:1, S * hh:S * (hh + 1)],
                                        start=True, stop=True,
                                    )
                                bcb = norm_pool.tile([128, S], BF16, tag="bcb",
                                                     name="bcb")
                                nc.vector.tensor_copy(out=bcb, in_=bc)
                                nc.vector.tensor_tensor(
                                    avn_sb[:, pp, :], avn_sb[:, pp, :], bcb,
                                    ALU.mult,
                                )

                        # mid-attention partial output projection: after pair
                        # PROJ_SPLIT-1 completes, normalize + project pairs
                        # 0..PROJ_SPLIT-1 while later pairs' attention runs
                        if phases == "all" and p == PROJ_SPLIT - 1 and PROJ_SPLIT > 0:
                            finish_pairs(range(PROJ_SPLIT))
                            for n in range(KT):
                                pps = st_ps_pool.tile(
                                    [128, ST_GROUP * 512], F32,
                                    tag=f"stg{n % N_GROUPS}", name="projps"
                                )
                                for kt in range(PROJ_SPLIT):
                                    nc.tensor.matmul(
                                        pps[:, 0:S],
                                        lhsT=wo_sb[:, kt, 128 * n:128 * (n + 1)],
                                        rhs=avn_sb[:, kt, :],
                                        start=(kt == 0),
                                        stop=(kt == PROJ_SPLIT - 1),
                                    )
                                nc.vector.tensor_copy(
                                    out=yh_sb[:, n, :], in_=pps[:, 0:S]
                                )
                        if p == HEADS // 2 - 1:
                            finish_pairs(range(PROJ_SPLIT if phases == "all"
                                               else 0, HEADS // 2))

            # ============ Stage D: output projection (remaining pairs) =====
            if phases == "all":
              with (
                tc.tile_pool(name="out_ps", bufs=4, space="PSUM") as out_ps_pool,
                tc.tile_pool(name="out_sb", bufs=3) as out_sb_pool,
            ):
                for n in range(KT):
                    y_t = out_sb_pool.tile([128, S], F32, tag="yt", name="yt")
                    if PROJ_SPLIT < KT:
                        ps = out_ps_pool.tile([128, S], F32, tag="outps",
                                              name="outps")
                        for kt in range(PROJ_SPLIT, KT):
                            nc.tensor.matmul(
                                ps,
                                lhsT=wo_sb[:, kt, 128 * n:128 * (n + 1)],
                                rhs=avn_sb[:, kt, :],
                                start=(kt == PROJ_SPLIT),
                                stop=(kt == KT - 1),
                            )
                        if PROJ_SPLIT > 0:
                            nc.vector.tensor_tensor(
                                y_t, ps, yh_sb[:, n, :], ALU.add
                            )
                        else:
                            nc.vector.tensor_copy(out=y_t, in_=ps)
                    else:
                        nc.vector.tensor_copy(out=y_t, in_=yh_sb[:, n, :])
                    nc.sync.dma_start(
                        out_ap.rearrange("(n p) i -> n p i", p=128)[n], y_t
                    )

    nc.compile()
    return nc


_GRAPH_CACHE = {}


def _get_graph():
    if "nc" not in _GRAPH_CACHE:
        _GRAPH_CACHE["nc"] = build_graph()
    return _GRAPH_CACHE["nc"]


def make_in_maps(x, ln_scale, ln_bias, w_qkv, w_out):
    import ml_dtypes

    W = (np.asarray(ln_scale, np.float32)[:, None]
         * np.asarray(w_qkv, np.float32)).astype(ml_dtypes.bfloat16)
    cvec = (np.asarray(ln_bias, np.float32) @ np.asarray(w_qkv, np.float32)).astype(
        np.float32
    )
    wo = np.asarray(w_out, np.float32).astype(ml_dtypes.bfloat16)
    x = np.asarray(x, np.float32)
    return [
        {
            "x": np.ascontiguousarray(x[c * S:(c + 1) * S]),
            "wq": W,
            "cvec": cvec,
            "wo": wo,
        }
        for c in range(NC_CORES)
    ]


def kernel(x, ln_scale, ln_bias, w_qkv, w_out, b_out, **run_kwargs):
    in_maps = make_in_maps(x, ln_scale, ln_bias, w_qkv, w_out)
    nc = _get_graph()
    res = run_bass_kernel_spmd(
        nc, in_maps, core_ids=list(range(NC_CORES)), **run_kwargs
    )
    _GRAPH_CACHE["last_results"] = res
    out = np.concatenate(
        [np.asarray(r["out"]).T for r in res.results], axis=0
    )
    return (out + np.asarray(b_out, np.float32)).astype(np.float32)

